# revision 17
# baseline (speedup 1.0000x reference)
"""Trainium2 Bass kernel for DSDM cosine-softmin retrieval.

Computes, for a bank A [N, D] and query q [D]:
    sims      = (A @ q) / (||A_r|| * ||q||)           per row r
    weights   = softmax(sims / T)      (== softmin of (1 - sims)/T)
    retrieved = weights @ A                            -> [D]

Sharding: A is split row-wise across 8 NeuronCores (N/8 rows each).
Each core makes a single pass over its shard:
  - DVE: fused multiply+reduce (tensor_tensor_reduce) -> row dots A_r . q
  - ACT: fused Square+accumulate -> row squared norms; per-group Ln/Exp
         epilogue converts (dots, sqnorm) -> w = exp((sim - 1)/T)
         (fixed-shift softmax: sims <= 1 so exponent <= 0, no max pass)
  - PE : per row-tile matmul with w as the stationary [128,1] operand
         accumulating the weighted sum into PSUM
Then an on-device AllReduce (8 cores) of [num (D floats) | den (1 float)]
and a divide produce the full output on every core.

Numerics notes:
  - exp((sim-1)/T) is in [e^-20, 1] for T=0.1 -> fp32 safe without the
    usual running-max correction, which is what makes one pass possible.
  - The reference's eps clamp max(|a||q|, 1e-8) is a no-op for these
    norms (~sqrt(2048)) and is omitted.
  - 1/||a|| is computed as exp(-0.5 ln(sqnorm)) because ACT's
    Rsqrt/Reciprocal are banned for accuracy in bass, and Ln/Exp live in
    one ACT table set (no table reload churn).
"""

import sys

import numpy as np

try:
    import concourse.bass as bass
except ImportError:  # fresh grading dir: repo not on sys.path
    sys.path.insert(0, "/opt/trn_rl_repo")
    import concourse.bass as bass

import concourse.bacc as bacc

from contextlib import ExitStack

from concourse import mybir
from concourse.bass_utils import run_bass_kernel_spmd
from concourse.tile import TileContext

F32 = mybir.dt.float32

N_ADDRESSES = 131072
D = 2048
N_CORES = 8
N_SHARD = N_ADDRESSES // N_CORES  # 16384 rows per core
P = 128                           # SBUF partitions = rows per tile
NT = N_SHARD // P                 # 128 row-tiles per core
G = 8                             # tiles per epilogue group
NG = NT // G                      # 16 groups
NCHUNK = D // P                   # 16 stationary [128,128] chunks per tile
TEMPERATURE = 0.1
INV_T = 1.0 / TEMPERATURE

# Collective payload: [128, 17] p-major -- cols 0..15 = weighted-sum partials
# (num[d] at [p, c] with d = c*128 + p), col 16 = per-partition den partials.
# AllReduce is elementwise, so any layout consistent across cores works.
CC_COLS = NCHUNK + 1


def _build_nc() -> bass.Bass:
    # Bacc (not plain Bass): its finalize() runs generate_event_semaphores,
    # which splits multi-sem waits into EventSemaphore chains -- walrus
    # encodes at most ONE sync wait per compute instruction.
    nc = bacc.Bacc(None, num_devices=N_CORES)

    a_dram = nc.dram_tensor("addresses", [N_SHARD, D], F32, kind="ExternalInput")
    q_dram = nc.dram_tensor("query_address", [1, D], F32, kind="ExternalInput")
    out_dram = nc.dram_tensor("out", [1, D], F32, kind="ExternalOutput")

    AF = mybir.ActivationFunctionType
    ALU = mybir.AluOpType

    with ExitStack() as ctx:
        tc = ctx.enter_context(TileContext(nc))
        singles = ctx.enter_context(tc.tile_pool(name="singles", bufs=1))
        a_pool = ctx.enter_context(tc.tile_pool(name="a_pool", bufs=G + 5))
        tmp_pool = ctx.enter_context(tc.tile_pool(name="tmp_pool", bufs=2))
        sq_pool = ctx.enter_context(tc.tile_pool(name="sq_pool", bufs=2))
        stats = ctx.enter_context(tc.tile_pool(name="stats", bufs=4))
        psum = ctx.enter_context(tc.tile_pool(name="psum", bufs=1, space="PSUM"))
        dram = ctx.enter_context(tc.tile_pool(name="dram", bufs=1, space="DRAM"))

        # ---- one-time setup -------------------------------------------------
        q_bcast = singles.tile([P, D], F32)
        q_ap = q_dram[:]
        nc.sync.dma_start(
            out=q_bcast[:],
            in_=bass.AP(tensor=q_ap.tensor, offset=q_ap.offset, ap=[[0, P], q_ap.ap[-1]]),
        )

        # ||q||^2 per partition (identical on all 128), then
        # beta = -0.5 * ln(||q||^2)  so that
        # exp(-0.5*ln(sqnorm) + beta) = 1/(||a|| * ||q||)
        q_sq_scratch = sq_pool.tile([P, D], F32)
        qsq = singles.tile([P, 1], F32)
        nc.scalar.activation(
            out=q_sq_scratch[:], in_=q_bcast[:], func=AF.Square, accum_out=qsq[:]
        )
        lq = singles.tile([P, 1], F32)
        nc.scalar.activation(out=lq[:], in_=qsq[:], func=AF.Ln)
        beta = singles.tile([P, 1], F32)
        nc.vector.tensor_scalar_mul(beta[:], lq[:], -0.5)

        ones_col = singles.tile([P, 1], F32)
        nc.vector.memset(ones_col[:], 1.0)

        neg_invt = singles.tile([P, 1], F32)
        nc.vector.memset(neg_invt[:], -INV_T)

        den_all = singles.tile([P, NG], F32)

        # PSUM accumulators: weighted sum [128 dcols, 16 chunks] in one bank,
        # plus a [1,1] denominator bank for the post-collective reduce.
        num_psum = psum.tile([P, NCHUNK], F32, name="num_psum", tag="num_psum")
        den_psum = psum.tile([1, 1], F32, name="den_psum", tag="den_psum")

        # ---- main pass over row-tiles --------------------------------------
        for g in range(NG):
            dots_g = stats.tile([P, G], F32, name=f"dots_{g}", tag="dots")
            sq_g = stats.tile([P, G], F32, name=f"sq_{g}", tag="sq")
            a_tiles = []
            for j in range(G):
                t = g * G + j
                a_tile = a_pool.tile([P, D], F32, name=f"a_{t}", tag="a")
                nc.sync.dma_start(out=a_tile[:], in_=a_dram[t * P : (t + 1) * P, :])
                a_tiles.append(a_tile)

                # dots[r] = sum_d A[r,d] * q[d]   (DVE, fused multiply+reduce;
                # scalar_tensor_tensor lowers to InstTensorScalarPtr which this
                # walrus supports, unlike InstTensorTensorReduce)
                ttmp = tmp_pool.tile([P, D], F32, name=f"ttmp_{t}", tag="ttmp")
                nc.vector.scalar_tensor_tensor(
                    out=ttmp[:],
                    in0=a_tile[:],
                    scalar=1.0,
                    in1=q_bcast[:],
                    op0=ALU.mult,
                    op1=ALU.mult,
                    accum_out=dots_g[:, j : j + 1],
                )
                # sqnorm[r] = sum_d A[r,d]^2      (ACT, fused)
                stmp = sq_pool.tile([P, D], F32, name=f"stmp_{t}", tag="stmp")
                nc.scalar.activation(
                    out=stmp[:],
                    in_=a_tile[:],
                    func=AF.Square,
                    accum_out=sq_g[:, j : j + 1],
                )

            # ---- group epilogue: w = exp((sim - 1)/T) ----------------------
            lns_g = stats.tile([P, G], F32, name=f"lns_{g}", tag="lns")
            nc.scalar.activation(out=lns_g[:], in_=sq_g[:], func=AF.Ln)
            u_g = stats.tile([P, G], F32, name=f"u_{g}", tag="u")
            nc.scalar.activation(
                out=u_g[:], in_=lns_g[:], func=AF.Exp, scale=-0.5, bias=beta[:]
            )
            sims_g = stats.tile([P, G], F32, name=f"sims_{g}", tag="sims")
            nc.vector.tensor_mul(sims_g[:], dots_g[:], u_g[:])
            w_g = stats.tile([P, G], F32, name=f"w_{g}", tag="w")
            nc.scalar.activation(
                out=w_g[:],
                in_=sims_g[:],
                func=AF.Exp,
                scale=INV_T,
                bias=neg_invt[:],
                accum_out=den_all[:, g : g + 1],
            )

            # ---- weighted sum on PE ----------------------------------------
            # A-chunk as the STATIONARY operand: LDWEIGHTS loads fp32 at
            # 1 col/cycle (no fp32 penalty), and the moving operand is the
            # tiny w column (N=1). The fp32 MOVING path would be 4 cyc/col.
            # out[dcol, 0] += sum_r A[r, dcol] * w[r]
            # NOTE: start/stop are PSUM *bank*-scoped, not region-scoped -- a
            # start=True mid-bank resets the whole bank's has_written bits
            # (verified on HW). Exactly one start (first mm) and one stop
            # (last mm) for the whole [128,16] accumulator.
            for j in range(G):
                t = g * G + j
                for c in range(NCHUNK):
                    nc.tensor.matmul(
                        num_psum[:, c : c + 1],
                        lhsT=a_tiles[j][:, c * P : (c + 1) * P],
                        rhs=w_g[:, j : j + 1],
                        start=(t == 0 and c == 0),
                        stop=(t == NT - 1 and c == NCHUNK - 1),
                    )

        # ---- finalize: all-reduce [num | den partials], divide -------------
        final_sb = singles.tile([P, CC_COLS], F32)
        nc.vector.tensor_copy(out=final_sb[:, 0:NCHUNK], in_=num_psum[:, :])
        nc.vector.reduce_sum(
            final_sb[:, NCHUNK : NCHUNK + 1], den_all[:], axis=mybir.AxisListType.X
        )

        cc_in = dram.tile([P, CC_COLS], F32, name="cc_in")
        cc_out = dram.tile([P, CC_COLS], F32, name="cc_out", addr_space="Shared")
        nc.sync.dma_start(out=cc_in[:], in_=final_sb[:])
        nc.gpsimd.collective_compute(
            "AllReduce",
            mybir.AluOpType.add,
            replica_groups=[list(range(N_CORES))],
            ins=[cc_in[:]],
            outs=[cc_out[:]],
        )

        ar_sb = singles.tile([P, CC_COLS], F32)
        nc.sync.dma_start(out=ar_sb[:], in_=cc_out[:])
        # den_total = sum over partitions of the all-reduced den partials
        nc.tensor.matmul(
            den_psum[:, :],
            lhsT=ones_col[:],
            rhs=ar_sb[:, NCHUNK : NCHUNK + 1],
            start=True,
            stop=True,
        )
        rden = singles.tile([1, 1], F32)
        nc.vector.reciprocal(out=rden[:], in_=den_psum[:, :])
        rden_b = singles.tile([P, 1], F32)
        nc.gpsimd.partition_broadcast(rden_b[:], rden[:])
        res_sb = singles.tile([P, NCHUNK], F32)
        nc.vector.tensor_scalar_mul(res_sb[:], ar_sb[:, 0:NCHUNK], rden_b[:])
        # out[d] = res_sb[d % 128, d // 128] -> strided DRAM AP [[1,128],[128,16]]
        o_ap = out_dram[:]
        nc.sync.dma_start(
            out=bass.AP(
                tensor=o_ap.tensor, offset=o_ap.offset, ap=[[1, P], [P, NCHUNK]]
            ),
            in_=res_sb[:],
        )

    return nc


_NC_CACHE: bass.Bass | None = None


def _get_nc() -> bass.Bass:
    global _NC_CACHE
    if _NC_CACHE is None:
        nc = _build_nc()
        if not nc.is_finalized():
            nc.finalize()  # Bacc: runs the wait-splitting/reg-alloc passes
        _NC_CACHE = nc
    return _NC_CACHE


def run(inputs: dict, **run_kwargs):
    """Run the SPMD kernel; returns (output [D] np.float32, BassKernelResults)."""
    addresses = np.asarray(inputs["addresses"], dtype=np.float32)
    query = np.asarray(inputs["query_address"], dtype=np.float32)
    assert addresses.shape == (N_ADDRESSES, D), addresses.shape
    assert query.shape == (D,), query.shape

    q2d = np.ascontiguousarray(query.reshape(1, D))
    in_maps = [
        {
            "addresses": np.ascontiguousarray(
                addresses[i * N_SHARD : (i + 1) * N_SHARD]
            ),
            "query_address": q2d,
        }
        for i in range(N_CORES)
    ]
    res = run_bass_kernel_spmd(_get_nc(), in_maps, list(range(N_CORES)), **run_kwargs)
    # Every core holds the full all-reduced result; take core 0's.
    out = np.asarray(res.results[0]["out"], dtype=np.float32).reshape(D)
    return out, res


def kernel(**inputs) -> np.ndarray:
    out, _ = run(inputs)
    return out


# revision 22
# speedup vs baseline: 1.9506x; 1.9506x over previous
"""Trainium2 Bass kernel for DSDM cosine-softmin retrieval.

Computes, for a bank A [N, D] and query q [D]:
    sims      = (A @ q) / (||A_r|| * ||q||)           per row r
    weights   = softmax(sims / T)      (== softmin of (1 - sims)/T)
    retrieved = weights @ A                            -> [D]

Sharding: A is split row-wise across 8 NeuronCores (N/8 rows each).
Each core makes a single pass over its shard:
  - DVE: fused multiply+reduce (tensor_tensor_reduce) -> row dots A_r . q
  - ACT: fused Square+accumulate -> row squared norms; per-group Ln/Exp
         epilogue converts (dots, sqnorm) -> w = exp((sim - 1)/T)
         (fixed-shift softmax: sims <= 1 so exponent <= 0, no max pass)
  - PE : per row-tile matmul with w as the stationary [128,1] operand
         accumulating the weighted sum into PSUM
Then an on-device AllReduce (8 cores) of [num (D floats) | den (1 float)]
and a divide produce the full output on every core.

Numerics notes:
  - exp((sim-1)/T) is in [e^-20, 1] for T=0.1 -> fp32 safe without the
    usual running-max correction, which is what makes one pass possible.
  - The reference's eps clamp max(|a||q|, 1e-8) is a no-op for these
    norms (~sqrt(2048)) and is omitted.
  - 1/||a|| is computed as exp(-0.5 ln(sqnorm)) because ACT's
    Rsqrt/Reciprocal are banned for accuracy in bass, and Ln/Exp live in
    one ACT table set (no table reload churn).
"""

import sys

import numpy as np

try:
    import concourse.bass as bass
except ImportError:  # fresh grading dir: repo not on sys.path
    sys.path.insert(0, "/opt/trn_rl_repo")
    import concourse.bass as bass

import concourse.bacc as bacc

from contextlib import ExitStack

from concourse import mybir
from concourse.bass_utils import run_bass_kernel_spmd
from concourse.tile import TileContext

F32 = mybir.dt.float32

N_ADDRESSES = 131072
D = 2048
N_CORES = 8
N_SHARD = N_ADDRESSES // N_CORES  # 16384 rows per core
P = 128                           # SBUF partitions = rows per tile
NT = N_SHARD // P                 # 128 row-tiles per core
G = 8                             # tiles per epilogue group
NG = NT // G                      # 16 groups
CHUNK = 512                       # PE moving free dim (one fp32 PSUM bank)
NCHUNK = D // CHUNK               # 4
TEMPERATURE = 0.1
INV_T = 1.0 / TEMPERATURE

CC_LEN = D + 4  # collective payload: [num(D) | den | pad]

# Tiles whose weighted-sum runs on DVE (scalar_tensor_tensor accumulate)
# instead of PE. PE's fp32 moving-operand matmul costs 4 cyc/col -> 3.46us
# per tile, just above the ~2.95us/tile DMA pace; offloading ~1.25 tiles
# per 8-tile group rebalances PE below the DMA roofline while using DVE's
# slack. (GpSimd can't help: it shares an exclusive-lock SBUF port with DVE.)
def _dve_offloaded(g: int, j: int) -> bool:
    return j == 3 or (j == 6 and g % 4 == 0)


def _build_nc() -> bass.Bass:
    # Bacc (not plain Bass): its finalize() runs generate_event_semaphores,
    # which splits multi-sem waits into EventSemaphore chains -- walrus
    # encodes at most ONE sync wait per compute instruction.
    nc = bacc.Bacc(None, num_devices=N_CORES)

    a_dram = nc.dram_tensor("addresses", [N_SHARD, D], F32, kind="ExternalInput")
    q_dram = nc.dram_tensor("query_address", [1, D], F32, kind="ExternalInput")
    out_dram = nc.dram_tensor("out", [1, D], F32, kind="ExternalOutput")

    AF = mybir.ActivationFunctionType
    ALU = mybir.AluOpType

    with ExitStack() as ctx:
        tc = ctx.enter_context(TileContext(nc))
        singles = ctx.enter_context(tc.tile_pool(name="singles", bufs=1))
        a_pool = ctx.enter_context(tc.tile_pool(name="a_pool", bufs=G + 5))
        tmp_pool = ctx.enter_context(tc.tile_pool(name="tmp_pool", bufs=2))
        sq_pool = ctx.enter_context(tc.tile_pool(name="sq_pool", bufs=2))
        stats = ctx.enter_context(tc.tile_pool(name="stats", bufs=4))
        psum = ctx.enter_context(tc.tile_pool(name="psum", bufs=1, space="PSUM"))
        dram = ctx.enter_context(tc.tile_pool(name="dram", bufs=1, space="DRAM"))

        # ---- one-time setup -------------------------------------------------
        q_bcast = singles.tile([P, D], F32)
        q_ap = q_dram[:]
        nc.sync.dma_start(
            out=q_bcast[:],
            in_=bass.AP(tensor=q_ap.tensor, offset=q_ap.offset, ap=[[0, P], q_ap.ap[-1]]),
        )

        # ||q||^2 per partition (identical on all 128), then
        # beta = -0.5 * ln(||q||^2)  so that
        # exp(-0.5*ln(sqnorm) + beta) = 1/(||a|| * ||q||)
        q_sq_scratch = sq_pool.tile([P, D], F32)
        qsq = singles.tile([P, 1], F32)
        nc.scalar.activation(
            out=q_sq_scratch[:], in_=q_bcast[:], func=AF.Square, accum_out=qsq[:]
        )
        lq = singles.tile([P, 1], F32)
        nc.scalar.activation(out=lq[:], in_=qsq[:], func=AF.Ln)
        beta = singles.tile([P, 1], F32)
        nc.vector.tensor_scalar_mul(beta[:], lq[:], -0.5)

        ones_col = singles.tile([P, 1], F32)
        nc.vector.memset(ones_col[:], 1.0)

        neg_invt = singles.tile([P, 1], F32)
        nc.vector.memset(neg_invt[:], -INV_T)

        den_all = singles.tile([P, NG], F32)

        # PSUM accumulators: weighted-sum chunks (one bank each) + denominator.
        num_psum = [
            psum.tile([1, CHUNK], F32, name=f"num_psum_{c}", tag=f"num_psum_{c}")
            for c in range(NCHUNK)
        ]
        den_psum = psum.tile([1, 1], F32, name="den_psum", tag="den_psum")

        # DVE-side weighted-sum accumulator (per-partition partials),
        # ping-pong because scalar_tensor_tensor reads acc and writes new acc.
        acc_pp = [
            singles.tile([P, D], F32, name=f"acc_pp_{i}", tag=f"acc_pp_{i}")
            for i in range(2)
        ]
        nc.vector.memset(acc_pp[0][:], 0.0)
        n_dve_acc = 0

        # ---- main pass over row-tiles --------------------------------------
        for g in range(NG):
            dots_g = stats.tile([P, G], F32, name=f"dots_{g}", tag="dots")
            sq_g = stats.tile([P, G], F32, name=f"sq_{g}", tag="sq")
            a_tiles = []
            for j in range(G):
                t = g * G + j
                a_tile = a_pool.tile([P, D], F32, name=f"a_{t}", tag="a")
                nc.sync.dma_start(out=a_tile[:], in_=a_dram[t * P : (t + 1) * P, :])
                a_tiles.append(a_tile)

                # dots[r] = sum_d A[r,d] * q[d]   (DVE, fused multiply+reduce;
                # scalar_tensor_tensor lowers to InstTensorScalarPtr which this
                # walrus supports, unlike InstTensorTensorReduce)
                ttmp = tmp_pool.tile([P, D], F32, name=f"ttmp_{t}", tag="ttmp")
                nc.vector.scalar_tensor_tensor(
                    out=ttmp[:],
                    in0=a_tile[:],
                    scalar=1.0,
                    in1=q_bcast[:],
                    op0=ALU.mult,
                    op1=ALU.mult,
                    accum_out=dots_g[:, j : j + 1],
                )
                # sqnorm[r] = sum_d A[r,d]^2      (ACT, fused)
                stmp = sq_pool.tile([P, D], F32, name=f"stmp_{t}", tag="stmp")
                nc.scalar.activation(
                    out=stmp[:],
                    in_=a_tile[:],
                    func=AF.Square,
                    accum_out=sq_g[:, j : j + 1],
                )

            # ---- group epilogue: w = exp((sim - 1)/T) ----------------------
            lns_g = stats.tile([P, G], F32, name=f"lns_{g}", tag="lns")
            nc.scalar.activation(out=lns_g[:], in_=sq_g[:], func=AF.Ln)
            u_g = stats.tile([P, G], F32, name=f"u_{g}", tag="u")
            nc.scalar.activation(
                out=u_g[:], in_=lns_g[:], func=AF.Exp, scale=-0.5, bias=beta[:]
            )
            sims_g = stats.tile([P, G], F32, name=f"sims_{g}", tag="sims")
            nc.vector.tensor_mul(sims_g[:], dots_g[:], u_g[:])
            w_g = stats.tile([P, G], F32, name=f"w_{g}", tag="w")
            nc.scalar.activation(
                out=w_g[:],
                in_=sims_g[:],
                func=AF.Exp,
                scale=INV_T,
                bias=neg_invt[:],
                accum_out=den_all[:, g : g + 1],
            )

            # ---- weighted sum: PE matmuls + DVE-offloaded tiles ------------
            # PE path: w column stationary [128,1], A moving [128,512] per
            # chunk, accumulating into [1,512] PSUM banks. (fp32 moving is
            # 4 cyc/col; that's why some tiles go to DVE instead.)
            # NOTE: PSUM start/stop are bank-scoped; each [1,512] bank gets
            # start on its first matmul, and stop later on the final
            # acc-reduce matmul after the DVE partials are folded in.
            for j in range(G):
                t = g * G + j
                if _dve_offloaded(g, j):
                    # acc_new[p, :] = A[p, :] * w[p] + acc_old[p, :]
                    src = acc_pp[n_dve_acc % 2]
                    dst = acc_pp[(n_dve_acc + 1) % 2]
                    nc.vector.scalar_tensor_tensor(
                        out=dst[:],
                        in0=a_tiles[j][:],
                        scalar=w_g[:, j : j + 1],
                        in1=src[:],
                        op0=ALU.mult,
                        op1=ALU.add,
                    )
                    n_dve_acc += 1
                    continue
                for c in range(NCHUNK):
                    nc.tensor.matmul(
                        num_psum[c][:, :],
                        lhsT=w_g[:, j : j + 1],
                        rhs=a_tiles[j][:, c * CHUNK : (c + 1) * CHUNK],
                        start=(t == 0),  # tile 0 is always a PE tile
                        stop=False,
                    )

        # Fold the DVE per-partition partials into the PSUM banks
        # (partition-reduce via ones-stationary matmul) and close the chains.
        acc_final = acc_pp[n_dve_acc % 2]
        for c in range(NCHUNK):
            nc.tensor.matmul(
                num_psum[c][:, :],
                lhsT=ones_col[:],
                rhs=acc_final[:, c * CHUNK : (c + 1) * CHUNK],
                start=False,
                stop=True,
            )

        # ---- finalize: den scalar, all-reduce [num | den], divide ----------
        den_col = singles.tile([P, 1], F32)
        nc.vector.reduce_sum(den_col[:], den_all[:], axis=mybir.AxisListType.X)
        nc.tensor.matmul(
            den_psum[:, :], lhsT=ones_col[:], rhs=den_col[:], start=True, stop=True
        )

        final_sb = singles.tile([1, CC_LEN], F32)
        nc.vector.memset(final_sb[:], 0.0)
        for c in range(NCHUNK):
            nc.vector.tensor_copy(
                out=final_sb[0:1, c * CHUNK : (c + 1) * CHUNK], in_=num_psum[c][:, :]
            )
        nc.vector.tensor_copy(out=final_sb[0:1, D : D + 1], in_=den_psum[:, :])

        cc_in = dram.tile([1, CC_LEN], F32, name="cc_in")
        cc_out = dram.tile([1, CC_LEN], F32, name="cc_out", addr_space="Shared")
        nc.sync.dma_start(out=cc_in[:], in_=final_sb[:])
        nc.gpsimd.collective_compute(
            "AllReduce",
            mybir.AluOpType.add,
            replica_groups=[list(range(N_CORES))],
            ins=[cc_in[:]],
            outs=[cc_out[:]],
        )

        ar_sb = singles.tile([1, CC_LEN], F32)
        nc.sync.dma_start(out=ar_sb[:], in_=cc_out[:])
        rden = singles.tile([1, 1], F32)
        nc.vector.reciprocal(out=rden[:], in_=ar_sb[0:1, D : D + 1])
        res_sb = singles.tile([1, D], F32)
        nc.vector.tensor_scalar_mul(res_sb[:], ar_sb[0:1, 0:D], rden[:])
        nc.sync.dma_start(out=out_dram[:], in_=res_sb[:])

    return nc


_NC_CACHE: bass.Bass | None = None


def _get_nc() -> bass.Bass:
    global _NC_CACHE
    if _NC_CACHE is None:
        nc = _build_nc()
        if not nc.is_finalized():
            nc.finalize()  # Bacc: runs the wait-splitting/reg-alloc passes
        _NC_CACHE = nc
    return _NC_CACHE


def run(inputs: dict, **run_kwargs):
    """Run the SPMD kernel; returns (output [D] np.float32, BassKernelResults)."""
    addresses = np.asarray(inputs["addresses"], dtype=np.float32)
    query = np.asarray(inputs["query_address"], dtype=np.float32)
    assert addresses.shape == (N_ADDRESSES, D), addresses.shape
    assert query.shape == (D,), query.shape

    q2d = np.ascontiguousarray(query.reshape(1, D))
    in_maps = [
        {
            "addresses": np.ascontiguousarray(
                addresses[i * N_SHARD : (i + 1) * N_SHARD]
            ),
            "query_address": q2d,
        }
        for i in range(N_CORES)
    ]
    res = run_bass_kernel_spmd(_get_nc(), in_maps, list(range(N_CORES)), **run_kwargs)
    # Every core holds the full all-reduced result; take core 0's.
    out = np.asarray(res.results[0]["out"], dtype=np.float32).reshape(D)
    return out, res


def kernel(**inputs) -> np.ndarray:
    out, _ = run(inputs)
    return out
